# revision 9
# baseline (speedup 1.0000x reference)
"""KNN group+gather kernel for Trainium2 (Bass/Tile), 8-core data parallel.

Problem: for each (b, g): find the 32 nearest xyz points to center[b, g]
(squared L2), gather them ordered by ascending distance, subtract the center.
  xyz    (16, 8192, 3) f32
  center (16, 512, 3)  f32
  out    (16, 512, 32, 3) f32

Sharding: batch 16 -> 8 cores x 2 batches. No cross-core communication.

Numerics: the grading reference runs eagerly on the same backend, computing
  dist = (c2 - 2*cx) + x2,  cx via an fp32 PE matmul,  then top_k(-dist).
This kernel reproduces that arithmetic bitwise:
  - scores s = 2c.x from a K=3 fp32 matmul with lhsT rows [2c0,2c1,2c2]
    (scaling weights by 2 commutes exactly with the fp32 pipeline),
  - u = (s - c2) - x2 on DVE in two rounded steps == -dist bitwise,
  - xyz is staged so transposed free position Q equals point index n, making
    every tie broken by ascending n exactly like a stable top_k.

Per-core flow (per batch b, per 128-center block gb):
  - stage xyz into SBUF [128, 64*3] with partition p holding points
    {j*128+p}, build rows [x0,x1,x2,|x|^2] via PE transposes -> XT [4, 8192]
    where free position Q == point index n.
  - x2rep [128, 8192] = row 3 replicated via K=1 ones-matmuls (exact).
  - per 512 tile: K=3 fp32 matmul -> PSUM; u = (psum - c2) - x2rep.
  - stage 1: top-16 (values + in-chunk positions) per 512 chunk via DVE
    max8 / max_index / match_replace (dup-tracking makes ties exact).
  - stage 2: top-32 of the 256 candidates; positions -> chunk-local index
    via an indirect-DMA gather through a DRAM scratch; n = chunk*512 + local.
  - indirect-DMA gather xyz[n], subtract center, store.

Host path: one packed fp32 input per core [2, 26112] (xyz|center rows), a
single shard_map'd bass_exec with no donated zero outputs, one D2H fetch.
"""

import sys

import numpy as np

try:
    import concourse.bass as bass  # noqa: F401
except ImportError:  # container default layout
    sys.path.insert(0, "/opt/trn_rl_repo")

import concourse.bass as bass
import concourse.bacc as bacc
import concourse.mybir as mybir
import concourse.tile as tile
from concourse.masks import make_identity

F32 = mybir.dt.float32
U32 = mybir.dt.uint32
ALU = mybir.AluOpType
NEG = -1.0e30

NCORES = 8
BPC = 2          # batches per core
N = 8192         # points
G = 512          # centers
M = 32           # neighbors
P = 128          # partitions
TW = 512         # free-dim tile width
NT = N // TW     # 16 tiles
JB = N // P      # 64 points per staging partition row
GB = G // P      # 4 center blocks
XYZF = N * 3     # 24576 xyz elements per batch
PACKW = XYZF + G * 3  # 26112 packed row width


def emit(ctx, tc, packed, out, scratch, dbg=None):
    nc = tc.nc

    const_pool = ctx.enter_context(tc.tile_pool(name="const", bufs=1))
    stage_pool = ctx.enter_context(tc.tile_pool(name="stage", bufs=2))
    xt_pool = ctx.enter_context(tc.tile_pool(name="xt", bufs=1))
    x2_pool = ctx.enter_context(tc.tile_pool(name="x2rep", bufs=1))
    c_pool = ctx.enter_context(tc.tile_pool(name="cmat", bufs=2))
    cst_pool = ctx.enter_context(tc.tile_pool(name="cst", bufs=8))
    u_pool = ctx.enter_context(tc.tile_pool(name="u", bufs=3))
    cand_pool = ctx.enter_context(tc.tile_pool(name="cand", bufs=2))
    sel_pool = ctx.enter_context(tc.tile_pool(name="sel", bufs=2))
    outp_pool = ctx.enter_context(tc.tile_pool(name="outp", bufs=2))
    ps_pool = ctx.enter_context(tc.tile_pool(name="ps", bufs=2, space="PSUM"))
    pst_pool = ctx.enter_context(tc.tile_pool(name="pst", bufs=2, space="PSUM"))

    identity = const_pool.tile([P, P], F32)
    make_identity(nc, identity[:])
    ones_col = const_pool.tile([1, P], F32)
    nc.vector.memset(ones_col[:], 1.0)
    # rowbase[p, i] = p * 256 (flat base of row p in the [128, 256] scratch)
    rowbase = const_pool.tile([P, M], U32)
    nc.gpsimd.iota(rowbase[:], pattern=[[0, M]], base=0, channel_multiplier=256)

    # views into the packed input
    xyz_rows = packed.rearrange("b (r d) -> (b r) d", d=3)  # [(BPC*8704), 3]
    out_v = out.rearrange("b g m d -> b g (m d)")           # [BPC, G, 96]
    scratch_flat = scratch.rearrange("s p c -> (s p c)")[:, None]

    for b in range(BPC):
        # ---- stage xyz with partition p holding points {j*128+p} so the
        # transposed free position Q equals the point index n ----
        staging = stage_pool.tile([P, JB * 3], F32)
        nc.sync.dma_start(
            staging[:].rearrange("p (j d) -> p j d", d=3),
            packed[b, 0:XYZF].rearrange("(j p d) -> p j d", p=P, d=3),
        )
        sq = stage_pool.tile([P, JB * 3], F32)
        nc.vector.tensor_mul(sq[:], staging[:], staging[:])
        staging2 = stage_pool.tile([P, JB * 4], F32)
        st2v = staging2[:].rearrange("p (j r) -> p j r", r=4)
        sqv = sq[:].rearrange("p (j d) -> p j d", d=3)
        stv = staging[:].rearrange("p (j d) -> p j d", d=3)
        nc.scalar.copy(st2v[:, :, 0:3], stv[:, :, :])
        nc.vector.tensor_add(st2v[:, :, 3:4], sqv[:, :, 0:1], sqv[:, :, 1:2])
        nc.vector.tensor_add(st2v[:, :, 3:4], st2v[:, :, 3:4], sqv[:, :, 2:3])

        # ---- transpose to XT [4, 8192]; free position Q == point n ----
        xt_all = xt_pool.tile([4, N], F32)
        for t in range(NT):
            pst = pst_pool.tile([4, TW], F32)
            for jj in range(4):
                j = 4 * t + jj
                nc.tensor.transpose(
                    pst[:, jj * P:(jj + 1) * P],
                    staging2[:, j * 4:(j + 1) * 4],
                    identity[:],
                )
            nc.scalar.copy(xt_all[:, t * TW:(t + 1) * TW], pst[:])

        # ---- x2 replicated across partitions via exact K=1 ones-matmul ----
        x2row = xt_pool.tile([1, N], F32, tag="x2row")
        nc.sync.dma_start(x2row[:], xt_all[3:4, :])
        x2rep = x2_pool.tile([P, N], F32)
        for t in range(NT):
            psx = pst_pool.tile([P, TW], F32, tag="psx")
            nc.tensor.matmul(
                psx[:],
                lhsT=ones_col[:],
                rhs=x2row[:, t * TW:(t + 1) * TW],
                start=True,
                stop=True,
            )
            nc.scalar.copy(x2rep[:, t * TW:(t + 1) * TW], psx[:])

        # ---- center blocks: cT rows [2c0, 2c1, 2c2] and c2 = |c|^2 ----
        cT_all = c_pool.tile([3, G], F32)
        cst3s = []
        c2s = []
        for gb in range(GB):
            cst3 = cst_pool.tile([P, 3], F32, tag=f"cst3_{b}_{gb}")
            nc.sync.dma_start(
                cst3[:],
                packed[b, XYZF:PACKW].rearrange("(g d) -> g d", d=3)[
                    gb * P:(gb + 1) * P, :
                ],
            )
            cst3s.append(cst3)
            csq = cst_pool.tile([P, 3], F32, tag="csq")
            nc.vector.tensor_mul(csq[:], cst3[:], cst3[:])
            c2 = cst_pool.tile([P, 1], F32, tag=f"c2_{b}_{gb}")
            nc.vector.tensor_add(c2[:], csq[:, 0:1], csq[:, 1:2])
            nc.vector.tensor_add(c2[:], c2[:], csq[:, 2:3])
            c2s.append(c2)
            cstage = cst_pool.tile([P, 3], F32, tag="cstage")
            nc.vector.tensor_scalar(
                cstage[:], cst3[:], 2.0, None, op0=ALU.mult
            )
            psc = pst_pool.tile([3, TW], F32, tag="psc")
            nc.tensor.transpose(psc[:, 0:P], cstage[:], identity[:])
            nc.scalar.copy(cT_all[:, gb * P:(gb + 1) * P], psc[:, 0:P])

        # ---- per center block: u = (2c.x - c2) - x2 (== -dist bitwise),
        # then two-level top-k with exact tie handling ----
        for gb in range(GB):
            cand_vals = cand_pool.tile([P, NT * 16], F32)
            cand_idx = cand_pool.tile([P, NT * 16], U32)
            for t in range(NT):
                ps = ps_pool.tile([P, TW], F32)
                nc.tensor.matmul(
                    ps[:],
                    lhsT=cT_all[:, gb * P:(gb + 1) * P],
                    rhs=xt_all[0:3, t * TW:(t + 1) * TW],
                    start=True,
                    stop=True,
                )
                ut = u_pool.tile([P, TW], F32, tag="ut")
                nc.vector.tensor_scalar(
                    ut[:], ps[:], c2s[gb][:], None, op0=ALU.subtract
                )
                nc.vector.tensor_tensor(
                    ut[:], ut[:], x2rep[:, t * TW:(t + 1) * TW],
                    op=ALU.subtract,
                )
                if dbg is not None and b == 0 and gb == 0 and t == 0:
                    dbs = u_pool.tile([P, TW], F32, tag="dbgs")
                    nc.scalar.copy(dbs[:], ps[:])
                    nc.sync.dma_start(dbg[0], dbs[:])
                    nc.sync.dma_start(dbg[1], ut[:])
                    dbx = u_pool.tile([P, TW], F32, tag="dbgx")
                    nc.vector.memset(dbx[:], 0.0)
                    nc.scalar.copy(dbx[0:4, :], xt_all[0:4, 0:TW])
                    nc.sync.dma_start(dbg[2], dbx[:])
                cv0 = cand_vals[:, 16 * t:16 * t + 8]
                ci0 = cand_idx[:, 16 * t:16 * t + 8]
                cv1 = cand_vals[:, 16 * t + 8:16 * t + 16]
                ci1 = cand_idx[:, 16 * t + 8:16 * t + 16]
                nc.vector.max(cv0, ut[:])
                nc.vector.max_index(ci0, cv0, ut[:])
                nc.vector.match_replace(
                    out=ut[:], in_to_replace=cv0, in_values=ut[:], imm_value=NEG
                )
                nc.vector.max(cv1, ut[:])
                nc.vector.max_index(ci1, cv1, ut[:])

            # stage 2: top-32 of the 256 candidates
            sel_vals = sel_pool.tile([P, M], F32)
            sel_pos = sel_pool.tile([P, M], U32)
            for r in range(4):
                sv = sel_vals[:, 8 * r:8 * r + 8]
                sp = sel_pos[:, 8 * r:8 * r + 8]
                nc.vector.max(sv, cand_vals[:])
                nc.vector.max_index(sp, sv, cand_vals[:])
                if r < 3:
                    nc.vector.match_replace(
                        out=cand_vals[:],
                        in_to_replace=sv,
                        in_values=cand_vals[:],
                        imm_value=NEG,
                    )

            # candidate position -> chunk-local index (gather via DRAM).
            # HW indirect DMA consumes ONE offset per partition with a
            # contiguous run, so gather each rank k with its own DMA.
            sidx = b * GB + gb
            nc.sync.dma_start(scratch[sidx], cand_idx[:])
            gpos = sel_pool.tile([P, M], U32)
            nc.vector.tensor_tensor(gpos[:], rowbase[:], sel_pos[:], op=ALU.add)
            qloc = sel_pool.tile([P, M], U32)
            for k in range(M):
                nc.gpsimd.indirect_dma_start(
                    out=qloc[:, k:k + 1],
                    out_offset=None,
                    in_=scratch_flat,
                    in_offset=bass.IndirectOffsetOnAxis(ap=gpos[:, k:k + 1], axis=0),
                    element_offset=sidx * P * 256,
                )
            # n = (sel_pos >> 4) * 512 + qloc  (free position == point index)
            nidx = sel_pool.tile([P, M], U32)
            nc.vector.tensor_scalar(
                nidx[:], sel_pos[:], 0xF0, 5,
                op0=ALU.bitwise_and, op1=ALU.logical_shift_left,
            )
            nc.vector.tensor_tensor(nidx[:], nidx[:], qloc[:], op=ALU.add)

            # gather the 32 neighbors (12 B each per partition per DMA)
            gath = outp_pool.tile([P, M * 3], F32)
            for k in range(M):
                nc.gpsimd.indirect_dma_start(
                    out=gath[:, 3 * k:3 * k + 3],
                    out_offset=None,
                    in_=xyz_rows,
                    in_offset=bass.IndirectOffsetOnAxis(ap=nidx[:, k:k + 1], axis=0),
                    element_offset=b * PACKW,
                )
            gv = gath[:].rearrange("p (m d) -> p m d", d=3)
            for d in range(3):
                nc.vector.tensor_scalar(
                    gv[:, :, d], gv[:, :, d], cst3s[gb][:, d:d + 1], None,
                    op0=ALU.subtract,
                )
            nc.sync.dma_start(out_v[b, gb * P:(gb + 1) * P, :], gath[:])


def build(debug=False):
    nc = bacc.Bacc("TRN2", target_bir_lowering=False, debug=False)
    packed = nc.dram_tensor("packed", [BPC, PACKW], F32, kind="ExternalInput")
    out = nc.dram_tensor("out", [BPC, G, M, 3], F32, kind="ExternalOutput")
    scratch = nc.dram_tensor("scratch", [BPC * GB, P, 256], U32, kind="Internal")
    dbg = None
    if debug:
        dbg = nc.dram_tensor("dbg", [3, P, TW], F32, kind="ExternalOutput")
    from contextlib import ExitStack

    with tile.TileContext(nc) as tc:
        with ExitStack() as ctx:
            emit(ctx, tc, packed.ap(), out.ap(), scratch.ap(),
                 dbg.ap() if debug else None)
    nc.compile()
    return nc


def _make_runner(nc, n_cores=NCORES, out_names=("out",)):
    """jit(shard_map(bass_exec)) mirroring run_bass_via_pjrt, but the
    output-shaped operands are persistent device-resident dummies (the PJRT
    plugin needs them to bind NEFF IO) with NO donation and NO per-call H2D;
    the kernel writes every output element, so results may start
    uninitialized. One packed input -> one H2D; one executable; one D2H."""
    import jax
    from jax.sharding import Mesh, PartitionSpec, NamedSharding
    from jax.experimental.shard_map import shard_map
    from concourse import bass2jax

    bass2jax.install_neuronx_cc_hook()

    out_shapes = {"out": (BPC, G, M, 3), "dbg": (3, P, TW)}
    out_avals = tuple(
        jax.core.ShapedArray(out_shapes[n], np.float32) for n in out_names
    )
    partition_name = (
        nc.partition_id_tensor.name if nc.partition_id_tensor else None
    )
    in_names = ("packed",) + tuple(out_names)
    if partition_name is not None:
        in_names = in_names + (partition_name,)

    def _body(packed_arr, *dummy_outs):
        operands = [packed_arr, *dummy_outs]
        if partition_name is not None:
            operands.append(bass2jax.partition_id_tensor())
        outs = bass2jax._bass_exec_p.bind(
            *operands,
            out_avals=out_avals,
            in_names=in_names,
            out_names=tuple(out_names),
            lowering_input_output_aliases=(),
            sim_require_finite=True,
            sim_require_nnan=True,
            nc=nc,
        )
        return tuple(outs)

    if n_cores == 1:
        dev = jax.devices()[0]
        dummies = tuple(
            jax.device_put(np.zeros(out_shapes[nm], np.float32), dev)
            for nm in out_names
        )

        def compile_fn():
            return (
                jax.jit(_body)
                .lower(
                    jax.ShapeDtypeStruct((BPC, PACKW), np.float32),
                    *(jax.ShapeDtypeStruct(out_shapes[nm], np.float32)
                      for nm in out_names),
                )
                .compile()
            )
    else:
        devices = jax.devices()[:n_cores]
        mesh = Mesh(np.asarray(devices), ("core",))
        sh = NamedSharding(mesh, PartitionSpec("core"))
        dummies = tuple(
            jax.device_put(
                np.zeros((n_cores * out_shapes[nm][0],) + out_shapes[nm][1:],
                         np.float32), sh)
            for nm in out_names
        )
        sharded = shard_map(
            _body,
            mesh=mesh,
            in_specs=(PartitionSpec("core"),) * (1 + len(out_names)),
            out_specs=tuple(PartitionSpec("core") for _ in out_names),
            check_rep=False,
        )

        def compile_fn():
            return (
                jax.jit(sharded)
                .lower(
                    jax.ShapeDtypeStruct((n_cores * BPC, PACKW), np.float32),
                    *(jax.ShapeDtypeStruct(d.shape, np.float32)
                      for d in dummies),
                )
                .compile()
            )

    compiled = bass2jax.fast_dispatch_compile(compile_fn)

    def run(packed_np):
        return compiled(packed_np, *dummies)

    return run


_RUNNER = None


def _pack(xyz, center):
    xyz = np.ascontiguousarray(xyz, dtype=np.float32)
    center = np.ascontiguousarray(center, dtype=np.float32)
    B = xyz.shape[0]
    return np.concatenate(
        [xyz.reshape(B, XYZF), center.reshape(B, G * 3)], axis=1
    )


def kernel(xyz, center, _trace=False):
    global _RUNNER
    if _RUNNER is None:
        _RUNNER = _make_runner(build())
    packed = _pack(xyz, center)
    outs = _RUNNER(packed)
    return np.asarray(outs[0])
